# revision 37
# baseline (speedup 1.0000x reference)
"""ConvTransformerEncoderLayer on 8 trn2 NeuronCores.

Sharding: pure data-parallel over batch (B=8 -> 1 batch element per core).
Each core runs the full layer for its batch element; no collectives.

v8 layout strategy (S=1024, D=512, H=8, hd=64, DFF=2048):
  - ALL inputs merged into ONE dram tensor "blob" per core (per-call operand
    count 13 -> 2; the PJRT/axon dispatch path pays a per-operand cost that
    dominated the old per-call time). partition_id input dropped
    (enable_partition_id=False; no collectives).
  - score path (X, wq/wk/wv/wo, Q, K, AVT) float32r: self-loading weights,
    full PE rate at free-dim 512, fp32-accurate scores. Value/FFN path
    (VTx, et, w1, xT, hT, w2) bf16: halves SBUF + DMA bytes; PSUM
    accumulates fp32 everywhere, so rel err stays ~1e-3 (gate 2e-2).
  - Q,K convs produce [c, s]; V conv produces V^T [t, c] (+ ones column per
    head) so AV emits av^T [d, s] with the softmax denominator as a psum row.
  - scores of a head PAIR (bases 0/64) issue back-to-back as 64x128 row
    tiles (T0/T8) -> concurrent on the PE array, ~2x scores throughput.
  - softmax without max-subtraction (scores are O(10), fp32 exp safe).
  - attention software-pipelined: up to 2 pending units (4 et slots); exp
    (Act engine) hides under conv/scores matmuls.
  - startup: X tiles alternate sync/scalar DMA queues ahead of all weights;
    wq/wk are ct_out-major so conv ct0 starts after 1/4 of the weight bytes;
    srcs has its own SBUF slot and streams on the gpsimd queue during
    attention; final AV units interleave with the first Wo/LN1 tiles.
  - LayerNorm normalize is one DVE tensor_scalar: (z-mu)*rstd; gamma/beta
    folded into W1/b1 host-side (device fixups only when nontrivial);
    FFN1 bias+relu on DVE (tensor_scalar add+max), not Act.
  - bo+Wo@bv folded into residual src host-side; b1+W1@be1 folded into b1;
    no bias matmuls anywhere.
  - SBUF slots are retagged across phases (X->xs, Q->xT, K->y, et->hT).
  - kernel() uses a cached jit(shard_map) executor (trace once per process).
"""
import sys

sys.path.insert(0, "/opt/trn_rl_repo")
import numpy as np

P = 128          # partitions
S = 1024         # sequence
D = 512          # d_model
H = 8            # heads
HD = 64          # head dim
DFF = 2048
KS = 3           # conv kernel size
EPS = 1e-5
NCORES = 8
CT = D // P      # 4 channel tiles
ST = S // P      # 8 sequence tiles
FT = DFF // P    # 16 ff tiles
SH = 512         # matmul free-dim chunk (= psum bank)
LAG = 2          # attention software-pipeline depth (pair units)

# blob layout: name -> (offset, length) in fp32 SLOTS per partition.
# Startup-critical regions first (DMA issue order follows blob order).
# w1/w2 are shipped bf16 (2 per fp32 slot); everything else fp32.
_BLOB_SPEC = [
    ("srcT", CT * S),          # 4096
    ("wv", CT * D),            # 2048
    ("wq", CT * KS * D),       # 6144
    ("wk", CT * KS * D),       # 6144
    ("bq", CT),
    ("bk", CT),
    ("ident", P),
    ("wo", CT * D),            # 2048
    ("src_sd", ST * D),        # 4096
    ("w1", CT * DFF // 2),     # 4096 (bf16)
    ("b1", FT),
    ("w2", FT * D // 2),       # 4096 (bf16)
]
_EXT_NAMES = ["g1r", "r1r", "g2r", "be2r"]  # appended when flags set

_CACHE = {}


def _blob_layout(flags):
    spec = list(_BLOB_SPEC)
    for name, fl in zip(_EXT_NAMES, flags):
        if fl:
            spec.append((name, D))
    off = {}
    pos = 0
    for name, ln in spec:
        off[name] = (pos, ln)
        pos += ln
    return off, pos


def _build_nc(flags):
    resid_mul, resid_add, out_mul, out_add = flags
    import concourse.tile as tile
    from concourse import bacc, mybir

    f32 = mybir.dt.float32
    f32r = mybir.dt.float32r
    bf16 = mybir.dt.bfloat16
    AF = mybir.ActivationFunctionType
    ALU = mybir.AluOpType

    nc = bacc.Bacc("TRN2", target_bir_lowering=False, debug=False,
                   enable_asserts=False, num_devices=NCORES,
                   enable_partition_id=False)

    off, total = _blob_layout(flags)
    blob = nc.dram_tensor("blob", [P, total], f32r, kind="ExternalInput").ap()

    def bsl(name, *shape, dt=None):
        o, ln = off[name]
        ap = blob[:, o:o + ln]
        if shape:
            dims = dict(zip("abc", shape))
            pat = " ".join("abc"[:len(shape)]) + " rest"
            ap = ap.rearrange(f"p ({pat}) -> p " + " ".join("abc"[:len(shape)])
                              + " rest", **dims)
        if dt is not None:
            ap = ap.bitcast(dt)
        return ap

    out_d = nc.dram_tensor("out", [P, ST, D], f32, kind="ExternalOutput").ap()

    with tile.TileContext(nc) as tc:
        with (
            tc.tile_pool(name="big", bufs=1) as big,
            tc.tile_pool(name="etp", bufs=4) as etp,
            tc.tile_pool(name="small", bufs=1) as small,
            tc.tile_pool(name="tmp", bufs=1) as tmp,
            tc.tile_pool(name="tiny", bufs=4) as tiny,
            tc.tile_pool(name="nrm", bufs=1) as nrm,
            tc.tile_pool(name="psp", bufs=8, space="PSUM") as psp,
        ):
            # ---------- small constants (gpsimd queue, tiny) ----------
            identity = small.tile([P, P], f32r, tag="ident")
            nc.gpsimd.dma_start(identity[:], bsl("ident"))
            bq_t = small.tile([P, CT], f32, tag="bq")
            nc.gpsimd.dma_start(bq_t[:], bsl("bq", dt=f32))
            bk_t = small.tile([P, CT], f32, tag="bk")
            nc.gpsimd.dma_start(bk_t[:], bsl("bk", dt=f32))
            b1_t = small.tile([P, FT], f32, tag="b1")
            nc.gpsimd.dma_start(b1_t[:], bsl("b1", dt=f32))
            ext_t = {}
            for k, fl in zip(_EXT_NAMES, flags):
                if not fl:
                    continue
                ext_t[k] = small.tile([P, D], f32, tag=k)
                nc.gpsimd.dma_start(ext_t[k][:], bsl(k, dt=f32))
            epsv = small.tile([P, 1], f32, tag="eps")
            nc.vector.memset(epsv[:], EPS)

            # ---------- bulk DMAs: startup-critical first ----------
            # X tiles alternate sync/scalar queues so no weight DMA can cut
            # ahead of the src data; conv weights are ct_out-major so each
            # ct-group's slice lands just in time for its convs.
            X = big.tile([P, CT, S + 2], f32r, tag="A", name="X")
            nc.vector.memset(X[:, :, 0:1].bitcast(f32), 0.0)
            nc.vector.memset(X[:, :, S + 1:S + 2].bitcast(f32), 0.0)
            srcT_v = bsl("srcT", CT)
            for ct in range(CT):
                eng = nc.sync if ct % 2 == 0 else nc.scalar
                eng.dma_start(X[:, ct, 1:S + 1], srcT_v[:, ct, :])
            wv = big.tile([P, CT, D], f32r, tag="WC", name="wv_s")
            wv_v = bsl("wv", CT)
            nc.sync.dma_start(wv[:, 0:2], wv_v[:, 0:2])
            nc.scalar.dma_start(wv[:, 2:4], wv_v[:, 2:4])
            # wq/wk blob layout: [P, ct_out, ci_t, k, 128]
            wq = big.tile([P, CT, CT, KS, P], f32r, tag="WA", name="wq_s")
            wk = big.tile([P, CT, CT, KS, P], f32r, tag="WB", name="wk_s")
            wq_v = bsl("wq", CT, CT, KS)
            wk_v = bsl("wk", CT, CT, KS)
            for ct in range(CT):
                nc.sync.dma_start(wq[:, ct], wq_v[:, ct])
                nc.scalar.dma_start(wk[:, ct], wk_v[:, ct])

            Q = big.tile([P, CT, S], f32r, tag="Q", name="Q")
            K = big.tile([P, CT, S], f32r, tag="K", name="K")
            VTx = big.tile([P, ST, H, HD + 1], bf16, tag="V", name="VTx")
            AVT = big.tile([P, CT, S], f32r, tag="AVT", name="AVT")
            # srcs has its OWN slot so its DMA isn't gated on VTx's death;
            # it streams in on the idle gpsimd queue during attention.
            srcs = big.tile([P, ST, D], f32, tag="SR", name="srcs")
            srcs_v = bsl("src_sd", ST, dt=f32)
            for half in range(2):
                nc.gpsimd.dma_start(srcs[:, 4 * half:4 * (half + 1)],
                                    srcs_v[:, 4 * half:4 * (half + 1)])

            # ---------- V conv -> VTx (V^T with a ones column per head) -----
            nc.vector.memset(VTx[:, :, :, HD:HD + 1], 1.0)
            for tt in range(ST):
                ps = psp.tile([P, SH], f32, tag="ps", bufs=6, name="psv")
                for ci in range(CT):
                    nc.tensor.matmul(ps[:], X[:, ci, 1 + tt * P:1 + (tt + 1) * P],
                                     wv[:, ci, :],
                                     start=(ci == 0), stop=(ci == CT - 1))
                nc.vector.tensor_copy(VTx[:, tt, :, 0:HD],
                                      ps.rearrange("p (h e) -> p h e", h=H))

            # wo into wv's slot (wv dead after V conv)
            wo = big.tile([P, CT, D], f32r, tag="WC", name="wo_s")
            nc.sync.dma_start(wo[:], bsl("wo", CT))

            # ---------- Q/K convs ----------
            def conv_qk(dst, w, bias_t, ct):
                for sc in range(2):
                    ps = psp.tile([P, SH], f32, tag="ps", bufs=6, name="psqk")
                    first = True
                    for ci in range(CT):
                        for k in range(KS):
                            nc.tensor.matmul(
                                ps[:], w[:, ct, ci, k, :],
                                X[:, ci, sc * SH + k: sc * SH + k + SH],
                                start=first, stop=(ci == CT - 1 and k == KS - 1))
                            first = False
                    nc.vector.tensor_scalar_add(
                        dst[:, ct, sc * SH:(sc + 1) * SH], ps[:],
                        bias_t[:, ct:ct + 1])

            # ---------- attention: paired scores+exp unit, AV unit ----------
            def scores_pair(ct, sc):
                """Both heads of ct (partition bases 0/64) as adjacent 64x128
                row-tiled matmuls -> concurrent on T0/T8."""
                ets = []
                pss = []
                for i in range(2):
                    et = etp.tile([P, ST, SH], bf16, tag="ET",
                                  name=f"et{2 * ct + i}_{sc}")
                    ets.append(et)
                for tt in range(ST):
                    for i in range(2):
                        base = HD * i
                        ps = psp.tile([P, SH], f32, tag="ps", bufs=6,
                                      name="pssc")
                        nc.tensor.matmul(
                            ps[:], K[base:base + HD, ct, tt * P:(tt + 1) * P],
                            Q[base:base + HD, ct, sc * SH:(sc + 1) * SH],
                            start=True, stop=True)
                        nc.scalar.activation(ets[i][:, tt, :], ps[:], AF.Exp,
                                             bias=0.0, scale=1.0 / HD)
                return ets

            def av_unit(h, sc, et):
                avps = psp.tile([P, SH], f32, tag="ps", bufs=6, name="avps")
                for tt in range(ST):
                    nc.tensor.matmul(avps[0:HD + 1, :], VTx[:, tt, h, :],
                                     et[:, tt, :],
                                     start=(tt == 0), stop=(tt == ST - 1))
                rrec = nrm.tile([1, SH], f32r, tag="rrec", name="rrec")
                with nc.allow_low_precision(reason="f32r softmax denom"):
                    nc.vector.reciprocal(rrec[0:1, :], avps[HD:HD + 1, :])
                rrep = nrm.tile([HD, SH], f32r, tag="rrep", name="rrep")
                nc.gpsimd.partition_broadcast(rrep[:], rrec[0:1, :])
                base_o = HD * (h % 2)
                nc.vector.tensor_tensor(
                    out=AVT[base_o:base_o + HD, h // 2, sc * SH:(sc + 1) * SH],
                    in0=avps[0:HD, :], in1=rrep[:], op=ALU.mult)

            # software pipeline: conv ct group, then paired scores; AV lags.
            # Drain pending to <=2 BEFORE allocating a pair's 2 et tiles so
            # the 4-slot ET pool never creates a circular WAR on the PE queue.
            pending = []
            for ct in range(CT):
                conv_qk(Q, wq, bq_t, ct)
                conv_qk(K, wk, bk_t, ct)
                for sc in range(2):
                    while len(pending) > 2:
                        av_unit(*pending.pop(0))
                    ets = scores_pair(ct, sc)
                    for i in range(2):
                        pending.append((2 * ct + i, sc, ets[i]))
            final_units = list(pending)
            pending = None

            # FFN weights (bf16) into the dead conv-weight slots
            w1 = big.tile([P, CT, DFF], bf16, tag="WA", name="w1_s")
            o1, l1 = off["w1"]
            nc.scalar.dma_start(w1[:], blob[:, o1:o1 + l1].bitcast(bf16)
                                .rearrange("p (a rest) -> p a rest", a=CT))
            w2 = big.tile([P, FT, D], bf16, tag="WB", name="w2_s")
            o2, l2 = off["w2"]
            nc.sync.dma_start(w2[:], blob[:, o2:o2 + l2].bitcast(bf16)
                              .rearrange("p (a rest) -> p a rest", a=FT))
            xs = big.tile([P, ST, D], f32r, tag="A", name="xs")   # X's slot
            xT = big.tile([P, CT, S], bf16, tag="Q", name="xT")   # Q's slot
            y = big.tile([P, ST, D], f32, tag="K", name="y")      # K's slot

            def ln_prep(z):
                """-> (mv, rstd) for DVE normalize of z [P, D]."""
                stats = tiny.tile([P, 6], f32, tag="st6", name="st6")
                nc.vector.bn_stats(stats[:], z[:])
                mv = tiny.tile([P, 2], f32, tag="mv", name="mv")
                nc.vector.bn_aggr(mv[:], stats[:])
                sd = tiny.tile([P, 1], f32, tag="sd", name="sd")
                nc.scalar.activation(sd[:], mv[:, 1:2], AF.Sqrt,
                                     bias=epsv[:], scale=1.0)
                rstd = tiny.tile([P, 1], f32, tag="rstd", name="rstd")
                nc.vector.reciprocal(rstd[:], sd[:])
                return mv, rstd

            # ---------- Wo projection + residual + LN1 ----------
            def wo_ln1(st):
                ps = psp.tile([P, SH], f32, tag="ps", bufs=6, name="pswo")
                for dt in range(CT):
                    nc.tensor.matmul(ps[:], AVT[:, dt, st * P:(st + 1) * P],
                                     wo[:, dt, :], start=(dt == 0),
                                     stop=(dt == CT - 1))
                z = tmp.tile([P, D], f32, tag="t1", bufs=1, name="z1")
                nc.vector.tensor_tensor(out=z[:], in0=ps[:],
                                        in1=srcs[:, st, :], op=ALU.add)
                mv, rstd = ln_prep(z)
                nc.vector.tensor_scalar(out=xs[:, st, :], in0=z[:],
                                        scalar1=mv[:, 0:1], scalar2=rstd[:],
                                        op0=ALU.subtract, op1=ALU.mult)

            def transpose_tile(st, dt):
                tp = psp.tile([P, P], f32r, tag="tp", bufs=2, name="tp")
                nc.tensor.transpose(tp[:], xs[:, st, dt * P:(dt + 1) * P],
                                    identity[:])
                nc.scalar.activation(xT[:, dt, st * P:(st + 1) * P],
                                     tp[:], AF.Copy)

            # Drain the last AV units interleaved with the first Wo/LN1 tiles:
            # once both sc=0 final AVs are done, all sc=0 AVT columns
            # (st 0..3) are complete, so their Wo matmuls fill PE while Act
            # finishes the sc=1 exps.
            av_unit(*final_units[0])
            av_unit(*final_units[1])
            wo_ln1(0)
            wo_ln1(1)
            av_unit(*final_units[2])
            wo_ln1(2)
            wo_ln1(3)
            av_unit(*final_units[3])
            for st in range(ST // 2):
                for dt in range(CT):
                    transpose_tile(st, dt)

            # ---------- FFN, overlapped with second-half Wo/LN1 ----------
            def ffn1_tile(sc, hT, ft):
                ps = psp.tile([P, SH], f32, tag="ps", bufs=6, name="psf1")
                for dt in range(CT):
                    nc.tensor.matmul(ps[:], w1[:, dt, ft * P:(ft + 1) * P],
                                     xT[:, dt, sc * SH:(sc + 1) * SH],
                                     start=(dt == 0), stop=(dt == CT - 1))
                nc.vector.tensor_scalar(out=hT[ft // 8][:, ft % 8, :],
                                        in0=ps[:],
                                        scalar1=b1_t[:, ft:ft + 1],
                                        scalar2=0.0,
                                        op0=ALU.add, op1=ALU.max)

            def ffn2_tile(sc, hT, j):
                st = sc * (ST // 2) + j
                ps = psp.tile([P, SH], f32, tag="ps", bufs=6, name="psf2")
                for ft in range(FT):
                    nc.tensor.matmul(
                        ps[:], hT[ft // 8][:, ft % 8, j * P:(j + 1) * P],
                        w2[:, ft, :], start=(ft == 0), stop=(ft == FT - 1))
                if resid_mul or resid_add:
                    xr = tmp.tile([P, D], f32, tag="xr", bufs=1, name="xr")
                    cur = xs[:, st, :]
                    if resid_mul:
                        nc.vector.tensor_tensor(out=xr[:], in0=cur,
                                                in1=ext_t["g1r"][:],
                                                op=ALU.mult)
                        cur = xr[:]
                    if resid_add:
                        nc.vector.tensor_tensor(out=xr[:], in0=cur,
                                                in1=ext_t["r1r"][:],
                                                op=ALU.add)
                    resid_ap = xr[:]
                else:
                    resid_ap = xs[:, st, :]
                z = tmp.tile([P, D], f32, tag="t1", bufs=1, name="z2")
                nc.vector.tensor_tensor(out=z[:], in0=ps[:],
                                        in1=resid_ap, op=ALU.add)
                mv, rstd = ln_prep(z)
                if out_mul or out_add:
                    yt = tmp.tile([P, D], f32, tag="t2", bufs=1, name="yt")
                    nc.vector.tensor_scalar(out=yt[:], in0=z[:],
                                            scalar1=mv[:, 0:1],
                                            scalar2=rstd[:],
                                            op0=ALU.subtract, op1=ALU.mult)
                    cur = yt[:]
                    if out_mul:
                        nc.vector.tensor_tensor(out=y[:, st, :], in0=cur,
                                                in1=ext_t["g2r"][:],
                                                op=ALU.mult)
                        cur = y[:, st, :]
                    if out_add:
                        nc.vector.tensor_tensor(out=y[:, st, :], in0=cur,
                                                in1=ext_t["be2r"][:],
                                                op=ALU.add)
                else:
                    nc.vector.tensor_scalar(out=y[:, st, :], in0=z[:],
                                            scalar1=mv[:, 0:1],
                                            scalar2=rstd[:],
                                            op0=ALU.subtract, op1=ALU.mult)
                nc.sync.dma_start(out_d[:, st, :], y[:, st, :])

            hT0 = [etp.tile([P, FT // 2, SH], bf16, tag="ET",
                            name=f"hT0_{i}") for i in range(2)]
            # interleave: second-half Wo/LN1 between first-half FFN1 tiles
            for ft in range(FT):
                ffn1_tile(0, hT0, ft)
                if ft < ST // 2:
                    wo_ln1(ST // 2 + ft)
                elif ft - ST // 2 < ST // 2:
                    st = ST // 2 + (ft - ST // 2)
                    for dt in range(CT):
                        transpose_tile(st, dt)
            hT1 = [etp.tile([P, FT // 2, SH], bf16, tag="ET",
                            name=f"hT1_{i}") for i in range(2)]
            # interleave: second-half FFN1 between first-half FFN2 tiles
            for j in range(ST // 2):
                ffn2_tile(0, hT0, j)
                for k in range(4):
                    ffn1_tile(1, hT1, 4 * j + k)
            for j in range(ST // 2):
                ffn2_tile(1, hT1, j)

    nc.compile()
    return nc


def _bf16_pack(a):
    """fp32 array [P, ...] -> bf16 bytes viewed as fp32 [P, n/2]."""
    import ml_dtypes
    b = np.ascontiguousarray(a.astype(ml_dtypes.bfloat16))
    return b.reshape(P, -1).view(np.uint16).view(np.float32)


def _prep_inputs(src, Wq, bq, Wk, bk, Wv, bv, Wo, bo, W1, b1, W2, b2,
                 g1, be1, g2, be2):
    f = np.float32

    def ctile(w):  # [co, ci(, k)] conv weight -> [p, ci_t(, k), co]
        wt = np.ascontiguousarray(np.moveaxis(w, 0, -1))  # [ci(,k), co]
        return np.ascontiguousarray(
            wt.reshape(CT, P, *wt.shape[1:]).transpose(1, 0, *range(2, wt.ndim + 1)))

    W1f = (W1 * np.asarray(g1)[None, :]).astype(f)      # fold gamma1
    b1f = (b1 + W1 @ be1).astype(f)                     # fold beta1
    r1 = (be1 + b2).astype(f)                           # residual additive fix
    flags = (not np.allclose(g1, 1.0), not np.allclose(r1, 0.0),
             not np.allclose(g2, 1.0), not np.allclose(be2, 0.0))

    def ctile_o(w):  # [co, ci, k] -> [P, co_t, ci_t, k, 128] (ct_out-major)
        a = ctile(w)                                     # [P, CT, KS, D]
        return np.ascontiguousarray(
            a.reshape(P, CT, KS, CT, P).transpose(0, 3, 1, 2, 4))

    pieces = {
        "wq": ctile_o(Wq).astype(f),                     # [P, CT, CT, KS, 128]
        "wk": ctile_o(Wk).astype(f),
        "wv": ctile(Wv[:, :, 0]).astype(f),              # [P, CT, D]
        "wo": np.ascontiguousarray(
            Wo.T.reshape(CT, P, D).transpose(1, 0, 2)).astype(f),
        "w1": _bf16_pack(np.ascontiguousarray(
            W1f.T.reshape(CT, P, DFF).transpose(1, 0, 2))),
        "w2": _bf16_pack(np.ascontiguousarray(
            W2.T.reshape(FT, P, D).transpose(1, 0, 2))),
        "bq": np.ascontiguousarray(bq.reshape(CT, P).T).astype(f),
        "bk": np.ascontiguousarray(bk.reshape(CT, P).T).astype(f),
        "b1": np.ascontiguousarray(b1f.reshape(FT, P).T).astype(f),
        "ident": np.eye(P, dtype=f),
    }
    if flags[0]:
        pieces["g1r"] = np.ascontiguousarray(np.broadcast_to(g1, (P, D))).astype(f)
    if flags[1]:
        pieces["r1r"] = np.ascontiguousarray(np.broadcast_to(r1, (P, D))).astype(f)
    if flags[2]:
        pieces["g2r"] = np.ascontiguousarray(np.broadcast_to(g2, (P, D))).astype(f)
    if flags[3]:
        pieces["be2r"] = np.ascontiguousarray(np.broadcast_to(be2, (P, D))).astype(f)

    off, total = _blob_layout(flags)
    shared = np.zeros((P, total), f)
    for name, (o, ln) in off.items():
        if name in ("srcT", "src_sd"):
            continue
        shared[:, o:o + ln] = pieces[name].reshape(P, ln)

    bo2 = (bo + Wo @ bv).astype(f)                       # folded into residual
    o_srcT, l_srcT = off["srcT"]
    o_ssd, l_ssd = off["src_sd"]
    in_maps = []
    for b in range(NCORES):
        m = shared.copy()
        m[:, o_srcT:o_srcT + l_srcT] = np.ascontiguousarray(
            src[b].T.reshape(CT, P, S).transpose(1, 0, 2)).astype(f).reshape(P, l_srcT)
        m[:, o_ssd:o_ssd + l_ssd] = np.ascontiguousarray(
            (src[b] + bo2[None, :]).reshape(ST, P, D).transpose(1, 0, 2)
        ).astype(f).reshape(P, l_ssd)
        in_maps.append({"blob": m})
    return in_maps, flags


def get_nc(flags=(False, False, False, False)):
    if ("nc", flags) not in _CACHE:
        _CACHE[("nc", flags)] = _build_nc(flags)
    return _CACHE[("nc", flags)]


def _get_runner(nc):
    """Cached jit(shard_map(bass_exec)) executor: trace/compile once, then
    each kernel() call is device_put + execute (run_bass_kernel_spmd
    rebuilds its jit closure every call, paying a full re-trace)."""
    key = ("runner", id(nc))
    if key in _CACHE:
        return _CACHE[key]
    import jax
    from jax.sharding import Mesh, PartitionSpec, NamedSharding
    try:
        from jax.shard_map import shard_map
    except ImportError:
        from jax.experimental.shard_map import shard_map
    from concourse import bass2jax, mybir

    bass2jax.install_neuronx_cc_hook()
    out_shape = (P, ST, D)
    out_avals = (jax.core.ShapedArray(out_shape, np.float32),)

    def _body(blob_in, out_zero):
        outs = bass2jax._bass_exec_p.bind(
            blob_in, out_zero,
            out_avals=out_avals,
            in_names=("blob", "out"),
            out_names=("out",),
            lowering_input_output_aliases=(),
            sim_require_finite=True,
            sim_require_nnan=True,
            nc=nc,
        )
        return tuple(outs)

    devices = jax.devices()[:NCORES]
    mesh = Mesh(np.asarray(devices), ("core",))
    spec = PartitionSpec("core")
    f = jax.jit(
        shard_map(_body, mesh=mesh, in_specs=(spec, spec), out_specs=(spec,),
                  check_rep=False),
        keep_unused=True,
    )
    sharding = NamedSharding(mesh, spec)
    zeros = np.zeros((NCORES * P, ST, D), np.float32)
    _CACHE[key] = (f, sharding, zeros, jax)
    return _CACHE[key]


def kernel(**inputs):
    in_maps, flags = _prep_inputs(**{k: np.asarray(v) for k, v in inputs.items()})
    nc = get_nc(flags)
    try:
        f, sharding, zeros, jax = _get_runner(nc)
        blob_all = np.concatenate([m["blob"] for m in in_maps], axis=0)
        blob_dev = jax.device_put(blob_all, sharding)
        out_dev = jax.device_put(zeros, sharding)
        (out_all,) = f(blob_dev, out_dev)
        out_np = np.asarray(out_all).reshape(NCORES, P, ST, D)
        outs = [out_np[c].transpose(1, 0, 2).reshape(S, D)
                for c in range(NCORES)]
        return np.stack(outs).astype(np.float32)
    except Exception:
        from concourse.bass_utils import run_bass_kernel_spmd
        res = run_bass_kernel_spmd(nc, in_maps, core_ids=list(range(NCORES)))
        outs = [r["out"].transpose(1, 0, 2).reshape(S, D) for r in res.results]
        return np.stack(outs).astype(np.float32)
